# revision 3
# baseline (speedup 1.0000x reference)
"""Distributed multi-head attention layer for 8 TRN2 NeuronCores.

Problem (hardcoded):
    B=2, SQ=2048, SC=2048, SKV=4096, DIM=1024, H=16, HD=64
    q = x@Wq; k = cat(k_cache, x@Wk); v = cat(v_cache, x@Wv)
    out = softmax(q k^T/sqrt(HD) + mask*NEG) v @ Wo ; returns (out, k, v)

Sharding: 8 cores = 2 batches x 4 head-groups (Megatron tensor parallel).
Core c handles batch b=c//4, head group hg=c%4 (heads 4hg..4hg+3, dim
slice 256hg..256hg+256). Wq/Wk/Wv split column-wise, Wo row-wise; the
4 per-batch out partials are summed on the host during unshard (no device
collectives needed).

Per-core kernel layout choices (so nothing big is transposed on device
except x itself and the tiny ctx):
  - xT = x.T via PE transposes; qT/kT_new computed in [dims, seq] layout,
    v_new in natural [seq, dims] layout.
  - scores are computed TRANSPOSED (S^T[skv, sq]); exp on ScalarE with the
    1/sqrt(HD) scaling folded into the activation's free scale; no
    max-subtraction (scores are bounded; masked lanes become exactly 0).
  - mask applied multiplicatively post-exp: host ships (1-mask).T in bf16,
    VectorE multiplies in 2x bf16 mode.
  - AV: P^T (bf16, straight out of exp) is the matmul stationary operand,
    V the moving one -> ctx in natural layout at full PE utilization. V
    carries a leading ones column per head so column 0 of ctx accumulates
    the softmax denominator; rows are normalized afterwards (flash-style
    deferred normalization).
  - ctx transposed back by PE, out partial = ctxT.T @ Wo_s in bf16.
"""

import numpy as np
import ml_dtypes

import concourse.bacc as bacc
import concourse.mybir as mybir
import concourse.tile as tile
from concourse import bass_utils

B, SQ, SC, DIM, H = 2, 2048, 2048, 1024, 16
SKV = SQ + SC  # 4096
HD = DIM // H  # 64
HG = 4  # head groups (cores per batch)
GD = DIM // HG  # 256 dims per head group
GH = H // HG  # 4 heads per group
INV_SQRT_HD = 1.0 / float(np.sqrt(HD))

F32 = mybir.dt.float32
F32R = mybir.dt.float32r
BF16 = mybir.dt.bfloat16

NSQ = SQ // 128  # 16 sq chunks
NKV = SKV // 128  # 32 skv chunks
NKD = DIM // 128  # 8 contraction chunks for projections
NC_SC = SC // 128  # 16 cache chunks
STRIP = 256  # sq strip width for the attention stage
NSTRIP = SQ // STRIP  # 8
ACT_SPAN = 4  # skv chunks per exp instruction ([128, 1024] free)
VW = GH * 65  # 260: per-head 65-wide V slots (ones col first)

_compiled_nc = None


def build_kernel():
    nc = bacc.Bacc("TRN2", target_bir_lowering=False)

    # ---- per-core I/O (host-prepared shards) ----
    x_in = nc.declare_dram_parameter("x", [SQ, DIM], F32R, isOutput=False)
    # k_cache slice transposed on host: [2, 128, SC]; [j, p, s] = dim 128j+p
    ktc_in = nc.declare_dram_parameter("ktc", [2, 128, SC], F32R, isOutput=False)
    # v in per-head 65-wide slots (ones col first); cache rows filled by host,
    # new rows hold ones + zeros (values overwritten on device)
    vaug_in = nc.declare_dram_parameter("vaug", [SKV, VW], BF16, isOutput=False)
    maskt_in = nc.declare_dram_parameter("maskt", [SKV, SQ], BF16, isOutput=False)
    wq_in = nc.declare_dram_parameter("wq", [DIM, GD], F32R, isOutput=False)
    wk_in = nc.declare_dram_parameter("wk", [DIM, GD], F32R, isOutput=False)
    wv_in = nc.declare_dram_parameter("wv", [DIM, GD], F32R, isOutput=False)
    wo_in = nc.declare_dram_parameter("wo", [GD, DIM], BF16, isOutput=False)
    ident_in = nc.declare_dram_parameter("ident", [128, 128], F32R, isOutput=False)
    identb_in = nc.declare_dram_parameter("identb", [128, 128], BF16, isOutput=False)

    out_out = nc.declare_dram_parameter("out", [SQ, DIM], F32, isOutput=True)
    ktn_out = nc.declare_dram_parameter("ktn", [2, 128, SQ], F32R, isOutput=True)
    vn_out = nc.declare_dram_parameter("vn", [SQ, GD], F32, isOutput=True)

    with tile.TileContext(nc) as tc:
        with (
            tc.tile_pool(name="persist", bufs=1) as persist,
        ):
            qT = persist.tile([128, 2, SQ], F32R)  # 2 MB   [dims(2 heads), j, sq]
            kT = persist.tile([128, 2, SKV], F32R)  # 4 MB
            V = persist.tile([128, NKV, VW], BF16)  # 2.1 MB
            wo_sb = persist.tile([128, 2, DIM], BF16)  # 0.5 MB
            ident = persist.tile([128, 128], F32R)
            identb = persist.tile([128, 128], BF16)

            nc.sync.dma_start(out=ident[:], in_=ident_in[:])
            nc.sync.dma_start(out=identb[:], in_=identb_in[:])
            nc.sync.dma_start(
                out=kT[:, :, 0:SC], in_=ktc_in.ap().rearrange("j p s -> p j s")
            )
            nc.sync.dma_start(
                out=V[:], in_=vaug_in.ap().rearrange("(c p) f -> p c f", p=128)
            )
            nc.sync.dma_start(
                out=wo_sb[:], in_=wo_in.ap().rearrange("(j p) d -> p j d", p=128)
            )

            # ---- Phase 1: x transpose + projections ----
            with (
                tc.tile_pool(name="xt", bufs=1) as xt_pool,
                tc.tile_pool(name="xs", bufs=3) as xs_pool,
                tc.tile_pool(name="w", bufs=1) as w_pool,
                tc.tile_pool(name="pst", bufs=2, space="PSUM") as pst,
                tc.tile_pool(name="psp", bufs=2, space="PSUM") as psp,
                tc.tile_pool(name="stage", bufs=3) as stage,
            ):
                xT = xt_pool.tile([128, NKD, SQ], F32R)  # 8 MB, freed after phase 1
                wq_sb = w_pool.tile([128, NKD, GD], F32R)
                wk_sb = w_pool.tile([128, NKD, GD], F32R)
                wv_sb = w_pool.tile([128, NKD, GD], F32R)
                for w_sb, w_in in ((wq_sb, wq_in), (wk_sb, wk_in), (wv_sb, wv_in)):
                    nc.sync.dma_start(
                        out=w_sb[:],
                        in_=w_in.ap().rearrange("(c p) d -> p c d", p=128),
                    )

                # x.T: PE-transpose 128x128 blocks; batch the 8 dim-chunks of
                # one sq-chunk into one [128, 1024] psum tile -> one ACT copy
                for m in range(NSQ):
                    xtile = xs_pool.tile([128, DIM], F32R, tag="x")
                    nc.sync.dma_start(
                        out=xtile[:], in_=x_in[m * 128 : (m + 1) * 128, :]
                    )
                    ps = pst.tile([128, NKD * 128], F32R, tag="tp")
                    for c in range(NKD):
                        nc.tensor.transpose(
                            ps[:, c * 128 : (c + 1) * 128],
                            xtile[:, c * 128 : (c + 1) * 128],
                            ident[:],
                        )
                    nc.scalar.copy(
                        out=xT[:, :, m * 128 : (m + 1) * 128], in_=ps[:]
                    )

                # qT / kT_new (transposed layout):
                for w_sb, dstT, off in ((wq_sb, qT, 0), (wk_sb, kT, SC)):
                    for j in range(2):
                        for n in range(SQ // 512):
                            ps = psp.tile([128, 512], F32, tag="proj")
                            for c in range(NKD):
                                nc.tensor.matmul(
                                    ps[:],
                                    w_sb[:, c, j * 128 : (j + 1) * 128],
                                    xT[:, c, n * 512 : (n + 1) * 512],
                                    start=(c == 0),
                                    stop=(c == NKD - 1),
                                )
                            nc.vector.tensor_copy(
                                dstT[:, j, off + n * 512 : off + (n + 1) * 512],
                                ps[:],
                            )
                # k_new out (host transposes back)
                nc.sync.dma_start(
                    out=ktn_out.ap().rearrange("j p s -> p j s"), in_=kT[:, :, SC:]
                )

                # v_new (natural layout) + bf16 per-head slots for AV
                for m in range(NSQ):
                    ps = psp.tile([128, GD], F32, tag="vproj")
                    for c in range(NKD):
                        nc.tensor.matmul(
                            ps[:],
                            xT[:, c, m * 128 : (m + 1) * 128],
                            wv_sb[:, c, :],
                            start=(c == 0),
                            stop=(c == NKD - 1),
                        )
                    vst = stage.tile([128, GD], F32, tag="vst")
                    nc.vector.tensor_copy(vst[:], ps[:])
                    nc.sync.dma_start(
                        out=vn_out[m * 128 : (m + 1) * 128, :], in_=vst[:]
                    )
                    for h in range(GH):
                        nc.scalar.copy(
                            out=V[:, NC_SC + m, h * 65 + 1 : h * 65 + 65],
                            in_=ps[:, h * 64 : (h + 1) * 64],
                        )

            # ---- Phase 2: attention + out-projection, strip by strip ----
            with (
                tc.tile_pool(name="mask", bufs=2) as mask_pool,
                tc.tile_pool(name="pt", bufs=2) as pt_pool,
                tc.tile_pool(name="sc", bufs=2, space="PSUM") as sc_pool,
                tc.tile_pool(name="cx", bufs=2, space="PSUM") as cx_pool,
                tc.tile_pool(name="ctx", bufs=2) as ctx_pool,
                tc.tile_pool(name="ost", bufs=3) as ost_pool,
                tc.tile_pool(name="sm", bufs=4) as sm_pool,
            ):
                for s in range(NSTRIP):
                    s0 = s * STRIP
                    maskt = mask_pool.tile([128, NKV, STRIP], BF16, tag="mask")
                    nc.sync.dma_start(
                        out=maskt[:],
                        in_=maskt_in.ap().rearrange("(c p) q -> p c q", p=128)[
                            :, :, s0 : s0 + STRIP
                        ],
                    )
                    ctxT = ctx_pool.tile([128, 2, STRIP], BF16, tag="ctxT")
                    for h in range(GH):
                        hp = (h % 2) * 64  # partition offset of head in qT/kT
                        j = h // 2
                        pt = pt_pool.tile([128, NKV, STRIP], BF16, tag="pt")
                        for g in range(NKV // ACT_SPAN):
                            ps = sc_pool.tile(
                                [128, ACT_SPAN * STRIP], F32, tag="sc"
                            )
                            for ci in range(ACT_SPAN):
                                c = g * ACT_SPAN + ci
                                nc.tensor.matmul(
                                    ps[:, ci * STRIP : (ci + 1) * STRIP],
                                    kT[hp : hp + 64, j, c * 128 : (c + 1) * 128],
                                    qT[hp : hp + 64, j, s0 : s0 + STRIP],
                                    start=True,
                                    stop=True,
                                )
                            nc.scalar.activation(
                                pt[:, g * ACT_SPAN : (g + 1) * ACT_SPAN, :],
                                ps[:],
                                mybir.ActivationFunctionType.Exp,
                                scale=INV_SQRT_HD,
                            )
                        nc.vector.tensor_tensor(
                            pt[:], pt[:], maskt[:], mybir.AluOpType.mult
                        )
                        # AV: ctx[sq 128, 65] = sum_c P^T[:,c,m].T @ V[:,c,h]
                        for m in range(STRIP // 128):
                            cx = cx_pool.tile([128, 65], F32, tag="cx")
                            for c in range(NKV):
                                nc.tensor.matmul(
                                    cx[:],
                                    pt[:, c, m * 128 : (m + 1) * 128],
                                    V[:, c, h * 65 : (h + 1) * 65],
                                    start=(c == 0),
                                    stop=(c == NKV - 1),
                                )
                            rec = sm_pool.tile([128, 1], F32, tag="rec")
                            nc.vector.reciprocal(rec[:], cx[:, 0:1])
                            ctxn = sm_pool.tile([128, 64], BF16, tag="ctxn")
                            nc.vector.tensor_scalar_mul(ctxn[:], cx[:, 1:65], rec[:])
                            # transpose into ctxT at this head's partitions
                            ctp = cx_pool.tile([128, 128], BF16, tag="ctp")
                            nc.tensor.transpose(
                                ctp[hp : hp + 64, :], ctxn[:], identb[:]
                            )
                            nc.vector.tensor_copy(
                                ctxT[hp : hp + 64, j, m * 128 : (m + 1) * 128],
                                ctp[hp : hp + 64, :],
                            )
                    # out partial for this strip
                    for m in range(STRIP // 128):
                        for n in range(DIM // 512):
                            po = cx_pool.tile([128, 512], F32, tag="ctp")
                            for j in range(2):
                                nc.tensor.matmul(
                                    po[:],
                                    ctxT[:, j, m * 128 : (m + 1) * 128],
                                    wo_sb[:, j, n * 512 : (n + 1) * 512],
                                    start=(j == 0),
                                    stop=(j == 1),
                                )
                            ost = ost_pool.tile([128, 512], F32, tag="ost")
                            nc.scalar.copy(out=ost[:], in_=po[:])
                            nc.sync.dma_start(
                                out=out_out[
                                    s0 + m * 128 : s0 + (m + 1) * 128,
                                    n * 512 : (n + 1) * 512,
                                ],
                                in_=ost[:],
                            )

    nc.finalize()
    return nc


_W = {}


def _prep_inputs(x, k_cache, v_cache, mask):
    """Host-side sharding + layout prep. Returns in_maps for 8 cores."""
    ident = np.eye(128, dtype=np.float32)
    identb = np.eye(128, dtype=ml_dtypes.bfloat16)
    keep_t = [
        np.ascontiguousarray((1.0 - mask[b]).T.astype(ml_dtypes.bfloat16))
        for b in range(B)
    ]
    xs = [np.ascontiguousarray(x[b]) for b in range(B)]
    in_maps = []
    for c in range(8):
        b, hg = divmod(c, HG)
        sl = slice(hg * GD, (hg + 1) * GD)
        ktc = np.ascontiguousarray(k_cache[b, :, sl].T).reshape(2, 128, SC)
        vaug = np.zeros((SKV, VW), dtype=ml_dtypes.bfloat16)
        vaug[:, 0:VW:65] = 1.0
        vc = v_cache[b, :, sl].astype(ml_dtypes.bfloat16)
        for h in range(GH):
            vaug[:SC, h * 65 + 1 : h * 65 + 65] = vc[:, h * 64 : (h + 1) * 64]
        in_maps.append(
            {
                "x": xs[b],
                "ktc": ktc,
                "vaug": vaug,
                "maskt": keep_t[b],
                "wq": np.ascontiguousarray(_W["Wq"][:, sl]),
                "wk": np.ascontiguousarray(_W["Wk"][:, sl]),
                "wv": np.ascontiguousarray(_W["Wv"][:, sl]),
                "wo": np.ascontiguousarray(_W["Wo"][sl, :]).astype(
                    ml_dtypes.bfloat16
                ),
                "ident": ident,
                "identb": identb,
            }
        )
    return in_maps


def kernel(x, k_cache, v_cache, mask, Wq, bq, Wk, bk, Wv, bv, Wo, bo, _trace=False):
    global _compiled_nc
    x = np.asarray(x)
    k_cache = np.asarray(k_cache)
    v_cache = np.asarray(v_cache)
    mask = np.asarray(mask)
    _W.update(
        Wq=np.asarray(Wq), Wk=np.asarray(Wk), Wv=np.asarray(Wv), Wo=np.asarray(Wo)
    )

    if _compiled_nc is None:
        _compiled_nc = build_kernel()
    nc = _compiled_nc

    in_maps = _prep_inputs(x, k_cache, v_cache, mask)
    res = bass_utils.run_bass_kernel_spmd(
        nc, in_maps, core_ids=list(range(8)), trace=_trace
    )
    kernel.last_results = res

    out = np.zeros((B, SQ, DIM), dtype=np.float32)
    k = np.empty((B, SKV, DIM), dtype=np.float32)
    v = np.empty((B, SKV, DIM), dtype=np.float32)
    k[:, :SC, :] = k_cache
    v[:, :SC, :] = v_cache
    for c in range(8):
        b, hg = divmod(c, HG)
        sl = slice(hg * GD, (hg + 1) * GD)
        r = res.results[c]
        out[b] += r["out"]
        k[b, SC:, sl] = r["ktn"].reshape(GD, SQ).T
        v[b, SC:, sl] = r["vn"]
    # biases are structurally zero in this problem; added for contract parity
    out += np.asarray(bo)[None, None, :]
    k[:, SC:, :] += np.asarray(bk)[None, None, :]
    v[:, SC:, :] += np.asarray(bv)[None, None, :]
    return out, k, v


# revision 4
# speedup vs baseline: 1.0561x; 1.0561x over previous
"""Distributed multi-head attention layer for 8 TRN2 NeuronCores.

Problem (hardcoded):
    B=2, SQ=2048, SC=2048, SKV=4096, DIM=1024, H=16, HD=64
    q = x@Wq; k = cat(k_cache, x@Wk); v = cat(v_cache, x@Wv)
    out = softmax(q k^T/sqrt(HD) + mask*NEG) v @ Wo ; returns (out, k, v)

Sharding: 8 cores = 2 batches x 4 head-groups (Megatron tensor parallel).
Core c handles batch b=c//4, head group hg=c%4 (heads 4hg..4hg+3, dim slice
256hg..+256). Wq/Wk/Wv split column-wise, Wo row-wise; the 4 per-batch out
partials are summed on the host during unshard (no device collectives).

Kernel structure per core (all layouts picked to avoid transposing anything
big on device except x itself and the tiny ctx):
  - xT = x.T via PE transposes; qT/kT_new projected in [dims, seq] layout,
    v_new in natural [seq, dims] layout (f32r matmuls: full-rate fp32).
  - scores computed TRANSPOSED (S^T[skv, sq]) in sq-pairs of 512 columns to
    amortize the f32r weight-load; exp on ScalarE with 1/sqrt(HD) folded in;
    no max-subtraction (scores bounded; masked lanes become exactly 0).
  - multiplicative keep-mask (host ships (1-mask).T bf16) on VectorE in 2x.
  - AV: P^T (bf16, straight from exp) is the stationary operand, V moving
    -> ctx natural at full PE utilization; V carries a leading ones column
    per head so ctx col 0 accumulates the softmax denominator; rows
    normalized afterwards (deferred flash-style normalization).
  - ctx transposed back by PE; out partial = ctxT.T @ Wo_s in bf16.
"""

import numpy as np
import ml_dtypes

import concourse.bass as bass
import concourse.bacc as bacc
import concourse.mybir as mybir
import concourse.tile as tile
from concourse import bass_utils

B, SQ, SC, DIM, H = 2, 2048, 2048, 1024, 16
SKV = SQ + SC  # 4096
HD = DIM // H  # 64
HG = 4  # head groups (cores per batch)
GD = DIM // HG  # 256 dims per head group
GH = H // HG  # 4 heads per group
INV_SQRT_HD = 1.0 / float(np.sqrt(HD))

F32 = mybir.dt.float32
F32R = mybir.dt.float32r
BF16 = mybir.dt.bfloat16

NSQ = SQ // 128  # 16 sq chunks
NKV = SKV // 128  # 32 skv chunks
NKD = DIM // 128  # 8 contraction chunks for projections
NC_SC = SC // 128  # 16 cache chunks
PW = 512  # sq pair width for the attention stage
NPAIR = SQ // PW  # 4
MW = 256  # mask tile width
G = 2  # skv chunks per exp instruction ([128, 1024] psum span)
VW = GH * 65  # 260: per-head 65-wide V slots (ones col first)

SCORES_BF16 = False  # q/k + scores matmul in bf16 (faster LDW, small acc loss)
KT_DT = BF16 if SCORES_BF16 else F32R

_compiled_nc = None


def build_kernel():
    nc = bacc.Bacc("TRN2", target_bir_lowering=False)

    # ---- per-core I/O (host-prepared shards) ----
    x_in = nc.declare_dram_parameter("x", [SQ, DIM], F32R, isOutput=False)
    # k_cache slice transposed on host: [2, 128, SC]; [j, p, s] = dim 128j+p
    ktc_in = nc.declare_dram_parameter("ktc", [2, 128, SC], KT_DT, isOutput=False)
    # v in per-head 65-wide slots (ones col first); cache rows filled by host,
    # new rows hold ones + zeros (values overwritten on device)
    vaug_in = nc.declare_dram_parameter("vaug", [SKV, VW], BF16, isOutput=False)
    maskt_in = nc.declare_dram_parameter("maskt", [SKV, SQ], BF16, isOutput=False)
    wq_in = nc.declare_dram_parameter("wq", [DIM, GD], F32R, isOutput=False)
    wk_in = nc.declare_dram_parameter("wk", [DIM, GD], F32R, isOutput=False)
    wv_in = nc.declare_dram_parameter("wv", [DIM, GD], F32R, isOutput=False)
    wo_in = nc.declare_dram_parameter("wo", [GD, DIM], BF16, isOutput=False)
    ident_in = nc.declare_dram_parameter("ident", [128, 128], F32R, isOutput=False)
    identb_in = nc.declare_dram_parameter("identb", [128, 128], BF16, isOutput=False)

    out_out = nc.declare_dram_parameter("out", [SQ, DIM], F32, isOutput=True)
    ktn_out = nc.declare_dram_parameter("ktn", [2, 128, SQ], F32R, isOutput=True)
    vn_out = nc.declare_dram_parameter("vn", [SQ, GD], F32, isOutput=True)

    with tile.TileContext(nc) as tc:
        with tc.tile_pool(name="persist", bufs=1) as persist:
            qT = persist.tile([128, 2, SQ], KT_DT)
            kT = persist.tile([128, 2, SKV], KT_DT)
            V = persist.tile([128, NKV, VW], BF16)  # 2.1 MB
            wo_sb = persist.tile([128, 2, DIM], BF16)  # 0.5 MB
            ident = persist.tile([128, 128], F32R)
            identb = persist.tile([128, 128], BF16)

            nc.sync.dma_start(out=ident[:], in_=ident_in[:])
            nc.sync.dma_start(out=identb[:], in_=identb_in[:])

            # ---- Phase 1: x transpose + projections ----
            with (
                tc.tile_pool(name="xt", bufs=1) as xt_pool,
                tc.tile_pool(name="xs", bufs=3) as xs_pool,
                tc.tile_pool(name="w", bufs=1) as w_pool,
                tc.tile_pool(name="pst", bufs=2, space="PSUM") as pst,
                tc.tile_pool(name="psp", bufs=2, space="PSUM") as psp,
                tc.tile_pool(name="stage", bufs=3) as stage,
            ):
                xT = xt_pool.tile([128, NKD, SQ], F32R)  # 8 MB, phase-1 only

                # x.T first: PE-transpose 128x128 blocks; 8 dim-chunks of one
                # sq-chunk batched into one [128, 1024] psum tile -> one copy
                for m in range(NSQ):
                    xtile = xs_pool.tile([128, DIM], F32R, tag="x")
                    nc.sync.dma_start(
                        out=xtile[:], in_=x_in[m * 128 : (m + 1) * 128, :]
                    )
                    ps = pst.tile([128, NKD * 128], F32R, tag="tp")
                    for c in range(NKD):
                        nc.tensor.transpose(
                            ps[:, c * 128 : (c + 1) * 128],
                            xtile[:, c * 128 : (c + 1) * 128],
                            ident[:],
                        )
                    nc.scalar.copy(out=xT[:, :, m * 128 : (m + 1) * 128], in_=ps[:])

                wq_sb = w_pool.tile([128, NKD, GD], F32R)
                wk_sb = w_pool.tile([128, NKD, GD], F32R)
                wv_sb = w_pool.tile([128, NKD, GD], F32R)
                for w_sb, w_in in ((wq_sb, wq_in), (wk_sb, wk_in), (wv_sb, wv_in)):
                    nc.sync.dma_start(
                        out=w_sb[:],
                        in_=w_in.ap().rearrange("(c p) d -> p c d", p=128),
                    )
                nc.sync.dma_start(
                    out=kT[:, :, 0:SC], in_=ktc_in.ap().rearrange("j p s -> p j s")
                )
                nc.sync.dma_start(
                    out=V[:], in_=vaug_in.ap().rearrange("(c p) f -> p c f", p=128)
                )
                nc.sync.dma_start(
                    out=wo_sb[:], in_=wo_in.ap().rearrange("(j p) d -> p j d", p=128)
                )

                # qT / kT_new (transposed layout)
                for w_sb, dstT, off in ((wq_sb, qT, 0), (wk_sb, kT, SC)):
                    for j in range(2):
                        for n in range(SQ // 512):
                            ps = psp.tile([128, 512], F32, tag="proj")
                            for c in range(NKD):
                                nc.tensor.matmul(
                                    ps[:],
                                    w_sb[:, c, j * 128 : (j + 1) * 128],
                                    xT[:, c, n * 512 : (n + 1) * 512],
                                    start=(c == 0),
                                    stop=(c == NKD - 1),
                                )
                            nc.vector.tensor_copy(
                                dstT[:, j, off + n * 512 : off + (n + 1) * 512],
                                ps[:],
                            )
                            if dstT is kT and SCORES_BF16:
                                kst = stage.tile([128, 512], F32R, tag="kst")
                                nc.scalar.copy(out=kst[:], in_=ps[:])
                                nc.sync.dma_start(
                                    out=ktn_out[j, :, n * 512 : (n + 1) * 512],
                                    in_=kst[:],
                                )
                if not SCORES_BF16:
                    nc.sync.dma_start(
                        out=ktn_out.ap().rearrange("j p s -> p j s"),
                        in_=kT[:, :, SC:],
                    )

                # v_new (natural layout) + bf16 per-head slots for AV
                for m in range(NSQ):
                    ps = psp.tile([128, GD], F32, tag="vproj")
                    for c in range(NKD):
                        nc.tensor.matmul(
                            ps[:],
                            xT[:, c, m * 128 : (m + 1) * 128],
                            wv_sb[:, c, :],
                            start=(c == 0),
                            stop=(c == NKD - 1),
                        )
                    vst = stage.tile([128, GD], F32, tag="vst")
                    nc.vector.tensor_copy(vst[:], ps[:])
                    nc.sync.dma_start(
                        out=vn_out[m * 128 : (m + 1) * 128, :], in_=vst[:]
                    )
                    # one strided copy into the 4 per-head value slots
                    vslot = V[:, NC_SC + m, :]
                    vslot_ap = bass.AP(
                        tensor=vslot.tensor,
                        offset=vslot.offset + 1,
                        ap=[[VW * NKV, 128], [65, GH], [1, 64]],
                    )
                    nc.scalar.copy(out=vslot_ap, in_=ps[:])

            # ---- Phase 2: attention + out-projection, sq-pair by sq-pair ----
            with (
                tc.tile_pool(name="mask", bufs=3) as mask_pool,
                tc.tile_pool(name="pt", bufs=2) as pt_pool,
                tc.tile_pool(name="sc", bufs=2, space="PSUM") as sc_pool,
                tc.tile_pool(name="cx", bufs=2, space="PSUM") as cx_pool,
                tc.tile_pool(name="ctx", bufs=2) as ctx_pool,
                tc.tile_pool(name="ost", bufs=3) as ost_pool,
                tc.tile_pool(name="sm", bufs=4) as sm_pool,
            ):
                for p in range(NPAIR):
                    p0 = p * PW
                    masks = []
                    for q in range(PW // MW):
                        mt = mask_pool.tile([128, NKV, MW], BF16, tag="mask")
                        nc.sync.dma_start(
                            out=mt[:],
                            in_=maskt_in.ap().rearrange("(c p) q -> p c q", p=128)[
                                :, :, p0 + q * MW : p0 + (q + 1) * MW
                            ],
                        )
                        masks.append(mt)
                    ctxT = ctx_pool.tile([128, 2, PW], BF16, tag="ctxT")
                    for h in range(GH):
                        hp = (h % 2) * 64
                        j = h // 2
                        pt = pt_pool.tile([128, NKV, PW], BF16, tag="pt")
                        for g in range(NKV // G):
                            ps = sc_pool.tile([128, G * PW], F32, tag="sc")
                            for ci in range(G):
                                c = g * G + ci
                                nc.tensor.matmul(
                                    ps[:, ci * PW : (ci + 1) * PW],
                                    kT[hp : hp + 64, j, c * 128 : (c + 1) * 128],
                                    qT[hp : hp + 64, j, p0 : p0 + PW],
                                    start=True,
                                    stop=True,
                                )
                            nc.scalar.activation(
                                pt[:, g * G : (g + 1) * G, :],
                                ps[:],
                                mybir.ActivationFunctionType.Exp,
                                scale=INV_SQRT_HD,
                            )
                        for q in range(PW // MW):
                            nc.vector.tensor_tensor(
                                pt[:, :, q * MW : (q + 1) * MW],
                                pt[:, :, q * MW : (q + 1) * MW],
                                masks[q][:],
                                mybir.AluOpType.mult,
                            )
                        # AV: ctx[sq 128, 65] = sum_c P^T[:,c,m].T @ V[:,c,h]
                        for m in range(PW // 128):
                            cx = cx_pool.tile([128, 65], F32, tag="cx")
                            for c in range(NKV):
                                nc.tensor.matmul(
                                    cx[:],
                                    pt[:, c, m * 128 : (m + 1) * 128],
                                    V[:, c, h * 65 : (h + 1) * 65],
                                    start=(c == 0),
                                    stop=(c == NKV - 1),
                                )
                            rec = sm_pool.tile([128, 1], F32, tag="rec")
                            nc.vector.reciprocal(rec[:], cx[:, 0:1])
                            ctxn = sm_pool.tile([128, 64], BF16, tag="ctxn")
                            nc.vector.tensor_scalar_mul(ctxn[:], cx[:, 1:65], rec[:])
                            ctp = cx_pool.tile([128, 128], BF16, tag="ctp")
                            nc.tensor.transpose(
                                ctp[hp : hp + 64, :], ctxn[:], identb[:]
                            )
                            nc.vector.tensor_copy(
                                ctxT[hp : hp + 64, j, m * 128 : (m + 1) * 128],
                                ctp[hp : hp + 64, :],
                            )
                    # out partial for this pair
                    for m in range(PW // 128):
                        for n in range(DIM // 512):
                            po = cx_pool.tile([128, 512], F32, tag="ctp")
                            for j in range(2):
                                nc.tensor.matmul(
                                    po[:],
                                    ctxT[:, j, m * 128 : (m + 1) * 128],
                                    wo_sb[:, j, n * 512 : (n + 1) * 512],
                                    start=(j == 0),
                                    stop=(j == 1),
                                )
                            ost = ost_pool.tile([128, 512], F32, tag="ost")
                            nc.vector.tensor_copy(ost[:], po[:])
                            nc.sync.dma_start(
                                out=out_out[
                                    p0 + m * 128 : p0 + (m + 1) * 128,
                                    n * 512 : (n + 1) * 512,
                                ],
                                in_=ost[:],
                            )

    nc.finalize()
    return nc


_W = {}


def _prep_inputs(x, k_cache, v_cache, mask):
    """Host-side sharding + layout prep. Returns in_maps for 8 cores."""
    ident = np.eye(128, dtype=np.float32)
    identb = np.eye(128, dtype=ml_dtypes.bfloat16)
    ktc_np = np.float32 if not SCORES_BF16 else ml_dtypes.bfloat16
    keep_t = [
        np.ascontiguousarray((1.0 - mask[b]).T.astype(ml_dtypes.bfloat16))
        for b in range(B)
    ]
    xs = [np.ascontiguousarray(x[b]) for b in range(B)]
    in_maps = []
    for c in range(8):
        b, hg = divmod(c, HG)
        sl = slice(hg * GD, (hg + 1) * GD)
        ktc = np.ascontiguousarray(
            k_cache[b, :, sl].T.astype(ktc_np)
        ).reshape(2, 128, SC)
        vaug = np.zeros((SKV, VW), dtype=ml_dtypes.bfloat16)
        vaug[:, 0:VW:65] = 1.0
        vc = v_cache[b, :, sl].astype(ml_dtypes.bfloat16)
        for h in range(GH):
            vaug[:SC, h * 65 + 1 : h * 65 + 65] = vc[:, h * 64 : (h + 1) * 64]
        in_maps.append(
            {
                "x": xs[b],
                "ktc": ktc,
                "vaug": vaug,
                "maskt": keep_t[b],
                "wq": np.ascontiguousarray(_W["Wq"][:, sl]),
                "wk": np.ascontiguousarray(_W["Wk"][:, sl]),
                "wv": np.ascontiguousarray(_W["Wv"][:, sl]),
                "wo": np.ascontiguousarray(_W["Wo"][sl, :]).astype(
                    ml_dtypes.bfloat16
                ),
                "ident": ident,
                "identb": identb,
            }
        )
    return in_maps


def kernel(x, k_cache, v_cache, mask, Wq, bq, Wk, bk, Wv, bv, Wo, bo, _trace=False):
    global _compiled_nc
    x = np.asarray(x)
    k_cache = np.asarray(k_cache)
    v_cache = np.asarray(v_cache)
    mask = np.asarray(mask)
    _W.update(
        Wq=np.asarray(Wq), Wk=np.asarray(Wk), Wv=np.asarray(Wv), Wo=np.asarray(Wo)
    )

    if _compiled_nc is None:
        _compiled_nc = build_kernel()
    nc = _compiled_nc

    in_maps = _prep_inputs(x, k_cache, v_cache, mask)
    res = bass_utils.run_bass_kernel_spmd(
        nc, in_maps, core_ids=list(range(8)), trace=_trace
    )
    kernel.last_results = res

    out = np.zeros((B, SQ, DIM), dtype=np.float32)
    k = np.empty((B, SKV, DIM), dtype=np.float32)
    v = np.empty((B, SKV, DIM), dtype=np.float32)
    k[:, :SC, :] = k_cache
    v[:, :SC, :] = v_cache
    for c in range(8):
        b, hg = divmod(c, HG)
        sl = slice(hg * GD, (hg + 1) * GD)
        r = res.results[c]
        out[b] += r["out"]
        k[b, SC:, sl] = r["ktn"].reshape(GD, SQ).T
        v[b, SC:, sl] = r["vn"]
    # biases are structurally zero in this problem; added for contract parity
    out += np.asarray(bo)[None, None, :]
    k[:, SC:, :] += np.asarray(bk)[None, None, :]
    v[:, SC:, :] += np.asarray(bv)[None, None, :]
    return out, k, v


# revision 5
# speedup vs baseline: 1.1426x; 1.0819x over previous
"""Distributed multi-head attention layer for 8 TRN2 NeuronCores.

Problem (hardcoded):
    B=2, SQ=2048, SC=2048, SKV=4096, DIM=1024, H=16, HD=64
    q = x@Wq; k = cat(k_cache, x@Wk); v = cat(v_cache, x@Wv)
    out = softmax(q k^T/sqrt(HD) + mask*NEG) v @ Wo ; returns (out, k, v)

Sharding: 8 cores = 2 batches x 4 head-groups (Megatron tensor parallel).
Core c handles batch b=c//4, head group hg=c%4 (heads 4hg..4hg+3, dim slice
256hg..+256). Wq/Wk/Wv split column-wise, Wo row-wise; the 4 per-batch out
partials are summed on the host during unshard (no device collectives).

Kernel structure per core (all layouts picked to avoid transposing anything
big on device except x itself and the tiny ctx):
  - xT = x.T via PE transposes; qT/kT_new projected in [dims, seq] layout,
    v_new in natural [seq, dims] layout (f32r matmuls: full-rate fp32).
  - scores computed TRANSPOSED (S^T[skv, sq]) in sq-pairs of 512 columns to
    amortize the f32r weight-load; exp on ScalarE with 1/sqrt(HD) folded in;
    no max-subtraction (scores bounded; masked lanes become exactly 0).
  - multiplicative keep-mask (host ships (1-mask).T bf16) on VectorE in 2x.
  - AV: P^T (bf16, straight from exp) is the stationary operand, V moving
    -> ctx natural at full PE utilization; V carries a leading ones column
    per head so ctx col 0 accumulates the softmax denominator; rows
    normalized afterwards (deferred flash-style normalization).
  - ctx transposed back by PE; out partial = ctxT.T @ Wo_s in bf16.
"""

import numpy as np
import ml_dtypes

import concourse.bass as bass
import concourse.bacc as bacc
import concourse.mybir as mybir
import concourse.tile as tile
from concourse import bass_utils

B, SQ, SC, DIM, H = 2, 2048, 2048, 1024, 16
SKV = SQ + SC  # 4096
HD = DIM // H  # 64
HG = 4  # head groups (cores per batch)
GD = DIM // HG  # 256 dims per head group
GH = H // HG  # 4 heads per group
INV_SQRT_HD = 1.0 / float(np.sqrt(HD))

F32 = mybir.dt.float32
F32R = mybir.dt.float32r
BF16 = mybir.dt.bfloat16

NSQ = SQ // 128  # 16 sq chunks
NKV = SKV // 128  # 32 skv chunks
NKD = DIM // 128  # 8 contraction chunks for projections
NC_SC = SC // 128  # 16 cache chunks
PW = 512  # sq pair width for the attention stage
NPAIR = SQ // PW  # 4
MW = 256  # mask tile width
G = 2  # skv chunks per exp instruction ([128, 1024] psum span)
VW = GH * 65  # 260: per-head 65-wide V slots (ones col first)

SCORES_BF16 = True  # q/k + scores matmul in bf16 (faster LDW, small acc loss)
KT_DT = BF16 if SCORES_BF16 else F32R

_compiled_nc = None


def build_kernel():
    nc = bacc.Bacc("TRN2", target_bir_lowering=False)

    # ---- per-core I/O (host-prepared shards) ----
    x_in = nc.declare_dram_parameter("x", [SQ, DIM], F32R, isOutput=False)
    # k_cache slice transposed on host: [2, 128, SC]; [j, p, s] = dim 128j+p
    ktc_in = nc.declare_dram_parameter("ktc", [2, 128, SC], KT_DT, isOutput=False)
    # v in per-head 65-wide slots (ones col first); cache rows filled by host,
    # new rows hold ones + zeros (values overwritten on device)
    vaug_in = nc.declare_dram_parameter("vaug", [SKV, VW], BF16, isOutput=False)
    maskt_in = nc.declare_dram_parameter("maskt", [SKV, SQ], BF16, isOutput=False)
    wq_in = nc.declare_dram_parameter("wq", [DIM, GD], F32R, isOutput=False)
    wk_in = nc.declare_dram_parameter("wk", [DIM, GD], F32R, isOutput=False)
    wv_in = nc.declare_dram_parameter("wv", [DIM, GD], F32R, isOutput=False)
    wo_in = nc.declare_dram_parameter("wo", [GD, DIM], BF16, isOutput=False)
    ident_in = nc.declare_dram_parameter("ident", [128, 128], F32R, isOutput=False)
    identb_in = nc.declare_dram_parameter("identb", [128, 128], BF16, isOutput=False)

    out_out = nc.declare_dram_parameter("out", [SQ, DIM], F32, isOutput=True)
    ktn_out = nc.declare_dram_parameter("ktn", [2, 128, SQ], F32R, isOutput=True)
    vn_out = nc.declare_dram_parameter("vn", [SQ, GD], F32, isOutput=True)

    with tile.TileContext(nc) as tc:
        with tc.tile_pool(name="persist", bufs=1) as persist:
            qT = persist.tile([128, 2, SQ], KT_DT)
            kT = persist.tile([128, 2, SKV], KT_DT)
            V = persist.tile([128, NKV, VW], BF16)  # 2.1 MB
            wo_sb = persist.tile([128, 2, DIM], BF16)  # 0.5 MB
            ident = persist.tile([128, 128], F32R)
            identb = persist.tile([128, 128], BF16)

            nc.sync.dma_start(out=ident[:], in_=ident_in[:])
            nc.sync.dma_start(out=identb[:], in_=identb_in[:])

            # ---- Phase 1: x transpose + projections ----
            with (
                tc.tile_pool(name="xt", bufs=1) as xt_pool,
                tc.tile_pool(name="xs", bufs=3) as xs_pool,
                tc.tile_pool(name="w", bufs=1) as w_pool,
                tc.tile_pool(name="pst", bufs=2, space="PSUM") as pst,
                tc.tile_pool(name="psp", bufs=2, space="PSUM") as psp,
                tc.tile_pool(name="stage", bufs=3) as stage,
            ):
                xT = xt_pool.tile([128, NKD, SQ], F32R)  # 8 MB, phase-1 only

                # x.T first: PE-transpose 128x128 blocks; 8 dim-chunks of one
                # sq-chunk batched into one [128, 1024] psum tile -> one copy
                for m in range(NSQ):
                    xtile = xs_pool.tile([128, DIM], F32R, tag="x")
                    nc.sync.dma_start(
                        out=xtile[:], in_=x_in[m * 128 : (m + 1) * 128, :]
                    )
                    ps = pst.tile([128, NKD * 128], F32R, tag="tp")
                    for c in range(NKD):
                        nc.tensor.transpose(
                            ps[:, c * 128 : (c + 1) * 128],
                            xtile[:, c * 128 : (c + 1) * 128],
                            ident[:],
                        )
                    nc.scalar.copy(out=xT[:, :, m * 128 : (m + 1) * 128], in_=ps[:])

                wq_sb = w_pool.tile([128, NKD, GD], F32R)
                wk_sb = w_pool.tile([128, NKD, GD], F32R)
                wv_sb = w_pool.tile([128, NKD, GD], F32R)
                for w_sb, w_in in ((wq_sb, wq_in), (wk_sb, wk_in), (wv_sb, wv_in)):
                    nc.sync.dma_start(
                        out=w_sb[:],
                        in_=w_in.ap().rearrange("(c p) d -> p c d", p=128),
                    )
                nc.sync.dma_start(
                    out=kT[:, :, 0:SC], in_=ktc_in.ap().rearrange("j p s -> p j s")
                )
                nc.sync.dma_start(
                    out=V[:], in_=vaug_in.ap().rearrange("(c p) f -> p c f", p=128)
                )
                nc.sync.dma_start(
                    out=wo_sb[:], in_=wo_in.ap().rearrange("(j p) d -> p j d", p=128)
                )

                # qT / kT_new (transposed layout)
                for w_sb, dstT, off in ((wq_sb, qT, 0), (wk_sb, kT, SC)):
                    for j in range(2):
                        for n in range(SQ // 512):
                            ps = psp.tile([128, 512], F32, tag="proj")
                            for c in range(NKD):
                                nc.tensor.matmul(
                                    ps[:],
                                    w_sb[:, c, j * 128 : (j + 1) * 128],
                                    xT[:, c, n * 512 : (n + 1) * 512],
                                    start=(c == 0),
                                    stop=(c == NKD - 1),
                                )
                            nc.vector.tensor_copy(
                                dstT[:, j, off + n * 512 : off + (n + 1) * 512],
                                ps[:],
                            )
                            if dstT is kT and SCORES_BF16:
                                kst = stage.tile([128, 512], F32R, tag="kst")
                                nc.scalar.copy(out=kst[:], in_=ps[:])
                                nc.sync.dma_start(
                                    out=ktn_out[j, :, n * 512 : (n + 1) * 512],
                                    in_=kst[:],
                                )
                if not SCORES_BF16:
                    nc.sync.dma_start(
                        out=ktn_out.ap().rearrange("j p s -> p j s"),
                        in_=kT[:, :, SC:],
                    )

                # v_new (natural layout) + bf16 per-head slots for AV
                for m in range(NSQ):
                    ps = psp.tile([128, GD], F32, tag="vproj")
                    for c in range(NKD):
                        nc.tensor.matmul(
                            ps[:],
                            xT[:, c, m * 128 : (m + 1) * 128],
                            wv_sb[:, c, :],
                            start=(c == 0),
                            stop=(c == NKD - 1),
                        )
                    vst = stage.tile([128, GD], F32, tag="vst")
                    nc.vector.tensor_copy(vst[:], ps[:])
                    nc.sync.dma_start(
                        out=vn_out[m * 128 : (m + 1) * 128, :], in_=vst[:]
                    )
                    # one strided copy into the 4 per-head value slots
                    vslot = V[:, NC_SC + m, :]
                    vslot_ap = bass.AP(
                        tensor=vslot.tensor,
                        offset=vslot.offset + 1,
                        ap=[[VW * NKV, 128], [65, GH], [1, 64]],
                    )
                    nc.scalar.copy(out=vslot_ap, in_=ps[:])

            # ---- Phase 2: attention + out-projection, sq-pair by sq-pair ----
            with (
                tc.tile_pool(name="mask", bufs=3) as mask_pool,
                tc.tile_pool(name="pt", bufs=2) as pt_pool,
                tc.tile_pool(name="sc", bufs=2, space="PSUM") as sc_pool,
                tc.tile_pool(name="cx", bufs=2, space="PSUM") as cx_pool,
                tc.tile_pool(name="ctx", bufs=2) as ctx_pool,
                tc.tile_pool(name="ost", bufs=3) as ost_pool,
                tc.tile_pool(name="sm", bufs=4) as sm_pool,
            ):
                for p in range(NPAIR):
                    p0 = p * PW
                    masks = []
                    for q in range(PW // MW):
                        mt = mask_pool.tile([128, NKV, MW], BF16, tag="mask")
                        nc.sync.dma_start(
                            out=mt[:],
                            in_=maskt_in.ap().rearrange("(c p) q -> p c q", p=128)[
                                :, :, p0 + q * MW : p0 + (q + 1) * MW
                            ],
                        )
                        masks.append(mt)
                    ctxT = ctx_pool.tile([128, 2, PW], BF16, tag="ctxT")
                    for h in range(GH):
                        hp = (h % 2) * 64
                        j = h // 2
                        pt = pt_pool.tile([128, NKV, PW], BF16, tag="pt")
                        for g in range(NKV // G):
                            ps = sc_pool.tile([128, G * PW], F32, tag="sc")
                            for ci in range(G):
                                c = g * G + ci
                                nc.tensor.matmul(
                                    ps[:, ci * PW : (ci + 1) * PW],
                                    kT[hp : hp + 64, j, c * 128 : (c + 1) * 128],
                                    qT[hp : hp + 64, j, p0 : p0 + PW],
                                    start=True,
                                    stop=True,
                                )
                            nc.scalar.activation(
                                pt[:, g * G : (g + 1) * G, :],
                                ps[:],
                                mybir.ActivationFunctionType.Exp,
                                scale=INV_SQRT_HD,
                            )
                        for q in range(PW // MW):
                            nc.vector.tensor_tensor(
                                pt[:, :, q * MW : (q + 1) * MW],
                                pt[:, :, q * MW : (q + 1) * MW],
                                masks[q][:],
                                mybir.AluOpType.mult,
                            )
                        # AV: ctx[sq 128, 65] = sum_c P^T[:,c,m].T @ V[:,c,h]
                        for m in range(PW // 128):
                            cx = cx_pool.tile([128, 65], F32, tag="cx")
                            for c in range(NKV):
                                nc.tensor.matmul(
                                    cx[:],
                                    pt[:, c, m * 128 : (m + 1) * 128],
                                    V[:, c, h * 65 : (h + 1) * 65],
                                    start=(c == 0),
                                    stop=(c == NKV - 1),
                                )
                            rec = sm_pool.tile([128, 1], F32, tag="rec")
                            nc.vector.reciprocal(rec[:], cx[:, 0:1])
                            ctxn = sm_pool.tile([128, 64], BF16, tag="ctxn")
                            nc.vector.tensor_scalar_mul(ctxn[:], cx[:, 1:65], rec[:])
                            ctp = cx_pool.tile([128, 128], BF16, tag="ctp")
                            nc.tensor.transpose(
                                ctp[hp : hp + 64, :], ctxn[:], identb[:]
                            )
                            nc.vector.tensor_copy(
                                ctxT[hp : hp + 64, j, m * 128 : (m + 1) * 128],
                                ctp[hp : hp + 64, :],
                            )
                    # out partial for this pair
                    for m in range(PW // 128):
                        for n in range(DIM // 512):
                            po = cx_pool.tile([128, 512], F32, tag="ctp")
                            for j in range(2):
                                nc.tensor.matmul(
                                    po[:],
                                    ctxT[:, j, m * 128 : (m + 1) * 128],
                                    wo_sb[:, j, n * 512 : (n + 1) * 512],
                                    start=(j == 0),
                                    stop=(j == 1),
                                )
                            ost = ost_pool.tile([128, 512], F32, tag="ost")
                            nc.vector.tensor_copy(ost[:], po[:])
                            nc.sync.dma_start(
                                out=out_out[
                                    p0 + m * 128 : p0 + (m + 1) * 128,
                                    n * 512 : (n + 1) * 512,
                                ],
                                in_=ost[:],
                            )

    nc.finalize()
    return nc


_W = {}


def _prep_inputs(x, k_cache, v_cache, mask):
    """Host-side sharding + layout prep. Returns in_maps for 8 cores."""
    ident = np.eye(128, dtype=np.float32)
    identb = np.eye(128, dtype=ml_dtypes.bfloat16)
    ktc_np = np.float32 if not SCORES_BF16 else ml_dtypes.bfloat16
    keep_t = [
        np.ascontiguousarray((1.0 - mask[b]).T.astype(ml_dtypes.bfloat16))
        for b in range(B)
    ]
    xs = [np.ascontiguousarray(x[b]) for b in range(B)]
    in_maps = []
    for c in range(8):
        b, hg = divmod(c, HG)
        sl = slice(hg * GD, (hg + 1) * GD)
        ktc = np.ascontiguousarray(
            k_cache[b, :, sl].T.astype(ktc_np)
        ).reshape(2, 128, SC)
        vaug = np.zeros((SKV, VW), dtype=ml_dtypes.bfloat16)
        vaug[:, 0:VW:65] = 1.0
        vc = v_cache[b, :, sl].astype(ml_dtypes.bfloat16)
        for h in range(GH):
            vaug[:SC, h * 65 + 1 : h * 65 + 65] = vc[:, h * 64 : (h + 1) * 64]
        in_maps.append(
            {
                "x": xs[b],
                "ktc": ktc,
                "vaug": vaug,
                "maskt": keep_t[b],
                "wq": np.ascontiguousarray(_W["Wq"][:, sl]),
                "wk": np.ascontiguousarray(_W["Wk"][:, sl]),
                "wv": np.ascontiguousarray(_W["Wv"][:, sl]),
                "wo": np.ascontiguousarray(_W["Wo"][sl, :]).astype(
                    ml_dtypes.bfloat16
                ),
                "ident": ident,
                "identb": identb,
            }
        )
    return in_maps


def kernel(x, k_cache, v_cache, mask, Wq, bq, Wk, bk, Wv, bv, Wo, bo, _trace=False):
    global _compiled_nc
    x = np.asarray(x)
    k_cache = np.asarray(k_cache)
    v_cache = np.asarray(v_cache)
    mask = np.asarray(mask)
    _W.update(
        Wq=np.asarray(Wq), Wk=np.asarray(Wk), Wv=np.asarray(Wv), Wo=np.asarray(Wo)
    )

    if _compiled_nc is None:
        _compiled_nc = build_kernel()
    nc = _compiled_nc

    in_maps = _prep_inputs(x, k_cache, v_cache, mask)
    res = bass_utils.run_bass_kernel_spmd(
        nc, in_maps, core_ids=list(range(8)), trace=_trace
    )
    kernel.last_results = res

    out = np.zeros((B, SQ, DIM), dtype=np.float32)
    k = np.empty((B, SKV, DIM), dtype=np.float32)
    v = np.empty((B, SKV, DIM), dtype=np.float32)
    k[:, :SC, :] = k_cache
    v[:, :SC, :] = v_cache
    for c in range(8):
        b, hg = divmod(c, HG)
        sl = slice(hg * GD, (hg + 1) * GD)
        r = res.results[c]
        out[b] += r["out"]
        k[b, SC:, sl] = r["ktn"].reshape(GD, SQ).T
        v[b, SC:, sl] = r["vn"]
    # biases are structurally zero in this problem; added for contract parity
    out += np.asarray(bo)[None, None, :]
    k[:, SC:, :] += np.asarray(bk)[None, None, :]
    v[:, SC:, :] += np.asarray(bv)[None, None, :]
    return out, k, v


# revision 6
# speedup vs baseline: 1.2815x; 1.1215x over previous
"""Distributed multi-head attention layer for 8 TRN2 NeuronCores.

Problem (hardcoded):
    B=2, SQ=2048, SC=2048, SKV=4096, DIM=1024, H=16, HD=64
    q = x@Wq; k = cat(k_cache, x@Wk); v = cat(v_cache, x@Wv)
    out = softmax(q k^T/sqrt(HD) + mask*NEG) v @ Wo ; returns (out, k, v)

Sharding: 8 cores = 2 batches x 4 head-groups (Megatron tensor parallel).
Core c handles batch b=c//4, head group hg=c%4 (heads 4hg..4hg+3, dim slice
256hg..+256). Wq/Wk/Wv split column-wise, Wo row-wise; the 4 per-batch out
partials are summed on the host during unshard (no device collectives).

Kernel structure per core (all layouts picked to avoid transposing anything
big on device except x itself and the tiny ctx):
  - xT = x.T via PE transposes; qT/kT_new projected in [dims, seq] layout,
    v_new in natural [seq, dims] layout (f32r matmuls: full-rate fp32).
  - scores computed TRANSPOSED (S^T[skv, sq]) in sq-pairs of 512 columns to
    amortize the f32r weight-load; exp on ScalarE with 1/sqrt(HD) folded in;
    no max-subtraction (scores bounded; masked lanes become exactly 0).
  - multiplicative keep-mask (host ships (1-mask).T bf16) on VectorE in 2x.
  - AV: P^T (bf16, straight from exp) is the stationary operand, V moving
    -> ctx natural at full PE utilization; V carries a leading ones column
    per head so ctx col 0 accumulates the softmax denominator; rows
    normalized afterwards (deferred flash-style normalization).
  - ctx transposed back by PE; out partial = ctxT.T @ Wo_s in bf16.
"""

import numpy as np
import ml_dtypes

import concourse.bass as bass
import concourse.bacc as bacc
import concourse.mybir as mybir
import concourse.tile as tile
from concourse import bass_utils

B, SQ, SC, DIM, H = 2, 2048, 2048, 1024, 16
SKV = SQ + SC  # 4096
HD = DIM // H  # 64
HG = 4  # head groups (cores per batch)
GD = DIM // HG  # 256 dims per head group
GH = H // HG  # 4 heads per group
INV_SQRT_HD = 1.0 / float(np.sqrt(HD))

F32 = mybir.dt.float32
F32R = mybir.dt.float32r
BF16 = mybir.dt.bfloat16

NSQ = SQ // 128  # 16 sq chunks
NKV = SKV // 128  # 32 skv chunks
NKD = DIM // 128  # 8 contraction chunks for projections
NC_SC = SC // 128  # 16 cache chunks
PW = 512  # sq pair width for the attention stage
NPAIR = SQ // PW  # 4
MW = 256  # mask tile width
G = 2  # skv chunks per exp instruction ([128, 1024] psum span)
VW = GH * 65  # 260: per-head 65-wide V slots (ones col first)

SCORES_BF16 = True  # q/k + scores matmul in bf16 (faster LDW, small acc loss)
KT_DT = BF16 if SCORES_BF16 else F32R

_compiled_nc = None


def build_kernel():
    nc = bacc.Bacc("TRN2", target_bir_lowering=False)

    # ---- per-core I/O (host-prepared shards) ----
    x_in = nc.declare_dram_parameter("x", [SQ, DIM], F32R, isOutput=False)
    # k_cache slice transposed on host, zero-padded per head to K=128:
    # [128, GH, SC]; head h occupies partitions (h%2)*64..+64, rest zero
    ktc_in = nc.declare_dram_parameter("ktc", [128, GH, SC], BF16, isOutput=False)
    # v in per-head 65-wide slots (ones col first); cache rows filled by host,
    # new rows hold ones + zeros (values overwritten on device)
    vaug_in = nc.declare_dram_parameter("vaug", [SKV, VW], BF16, isOutput=False)
    maskt_in = nc.declare_dram_parameter("maskt", [SKV, SQ], BF16, isOutput=False)
    wq_in = nc.declare_dram_parameter("wq", [DIM, GD], F32R, isOutput=False)
    wk_in = nc.declare_dram_parameter("wk", [DIM, GD], F32R, isOutput=False)
    wv_in = nc.declare_dram_parameter("wv", [DIM, GD], F32R, isOutput=False)
    wo_in = nc.declare_dram_parameter("wo", [GD, DIM], BF16, isOutput=False)
    ident_in = nc.declare_dram_parameter("ident", [128, 128], F32R, isOutput=False)
    identb_in = nc.declare_dram_parameter("identb", [128, 128], BF16, isOutput=False)

    out_out = nc.declare_dram_parameter("out", [SQ, DIM], F32, isOutput=True)
    ktn_out = nc.declare_dram_parameter("ktn", [2, 128, SQ], F32R, isOutput=True)
    vn_out = nc.declare_dram_parameter("vn", [SQ, GD], F32, isOutput=True)

    with tile.TileContext(nc) as tc:
        with tc.tile_pool(name="persist", bufs=1) as persist:
            qT = persist.tile([128, 2, SQ], KT_DT)
            kz = persist.tile([128, GH, SKV], BF16)  # 4 MB zero-padded kT
            V = persist.tile([128, NKV, VW], BF16)  # 2.1 MB
            wo_sb = persist.tile([128, 2, DIM], BF16)  # 0.5 MB
            ident = persist.tile([128, 128], F32R)
            identb = persist.tile([128, 128], BF16)

            nc.sync.dma_start(out=ident[:], in_=ident_in[:])
            nc.sync.dma_start(out=identb[:], in_=identb_in[:])

            # ---- Phase 1: x transpose + projections ----
            with (
                tc.tile_pool(name="xt", bufs=1) as xt_pool,
                tc.tile_pool(name="xs", bufs=3) as xs_pool,
                tc.tile_pool(name="w", bufs=1) as w_pool,
                tc.tile_pool(name="pst", bufs=2, space="PSUM") as pst,
                tc.tile_pool(name="psp", bufs=2, space="PSUM") as psp,
                tc.tile_pool(name="stage", bufs=3) as stage,
            ):
                xT = xt_pool.tile([128, NKD, SQ], F32R)  # 8 MB, phase-1 only

                # x.T first: PE-transpose 128x128 blocks; 8 dim-chunks of one
                # sq-chunk batched into one [128, 1024] psum tile -> one copy
                for m in range(NSQ):
                    xtile = xs_pool.tile([128, DIM], F32R, tag="x")
                    nc.sync.dma_start(
                        out=xtile[:], in_=x_in[m * 128 : (m + 1) * 128, :]
                    )
                    ps = pst.tile([128, NKD * 128], F32R, tag="tp")
                    for c in range(NKD):
                        nc.tensor.transpose(
                            ps[:, c * 128 : (c + 1) * 128],
                            xtile[:, c * 128 : (c + 1) * 128],
                            ident[:],
                        )
                    nc.scalar.copy(out=xT[:, :, m * 128 : (m + 1) * 128], in_=ps[:])

                wq_sb = w_pool.tile([128, NKD, GD], F32R)
                wk_sb = w_pool.tile([128, NKD, GD], F32R)
                wv_sb = w_pool.tile([128, NKD, GD], F32R)
                for w_sb, w_in in ((wq_sb, wq_in), (wk_sb, wk_in), (wv_sb, wv_in)):
                    nc.sync.dma_start(
                        out=w_sb[:],
                        in_=w_in.ap().rearrange("(c p) d -> p c d", p=128),
                    )
                nc.sync.dma_start(out=kz[:, :, 0:SC], in_=ktc_in.ap())
                nc.vector.memset(kz[:, :, SC:], 0.0)
                nc.sync.dma_start(
                    out=V[:], in_=vaug_in.ap().rearrange("(c p) f -> p c f", p=128)
                )
                nc.sync.dma_start(
                    out=wo_sb[:], in_=wo_in.ap().rearrange("(j p) d -> p j d", p=128)
                )

                # qT / kT_new (transposed layout)
                for w_sb, is_k in ((wq_sb, False), (wk_sb, True)):
                    for j in range(2):
                        for n in range(SQ // 512):
                            ps = psp.tile([128, 512], F32, tag="proj")
                            for c in range(NKD):
                                nc.tensor.matmul(
                                    ps[:],
                                    w_sb[:, c, j * 128 : (j + 1) * 128],
                                    xT[:, c, n * 512 : (n + 1) * 512],
                                    start=(c == 0),
                                    stop=(c == NKD - 1),
                                )
                            if not is_k:
                                nc.vector.tensor_copy(
                                    qT[:, j, n * 512 : (n + 1) * 512], ps[:]
                                )
                            else:
                                for hh in range(2):
                                    hp2 = hh * 64
                                    nc.vector.tensor_copy(
                                        kz[
                                            hp2 : hp2 + 64,
                                            2 * j + hh,
                                            SC + n * 512 : SC + (n + 1) * 512,
                                        ],
                                        ps[hp2 : hp2 + 64, :],
                                    )
                                kst = stage.tile([128, 512], F32R, tag="kst")
                                nc.scalar.copy(out=kst[:], in_=ps[:])
                                nc.sync.dma_start(
                                    out=ktn_out[j, :, n * 512 : (n + 1) * 512],
                                    in_=kst[:],
                                )

                # v_new (natural layout) + bf16 per-head slots for AV
                for m in range(NSQ):
                    ps = psp.tile([128, GD], F32, tag="vproj")
                    for c in range(NKD):
                        nc.tensor.matmul(
                            ps[:],
                            xT[:, c, m * 128 : (m + 1) * 128],
                            wv_sb[:, c, :],
                            start=(c == 0),
                            stop=(c == NKD - 1),
                        )
                    vst = stage.tile([128, GD], F32, tag="vst")
                    nc.vector.tensor_copy(vst[:], ps[:])
                    nc.sync.dma_start(
                        out=vn_out[m * 128 : (m + 1) * 128, :], in_=vst[:]
                    )
                    # one strided copy into the 4 per-head value slots
                    vslot = V[:, NC_SC + m, :]
                    vslot_ap = bass.AP(
                        tensor=vslot.tensor,
                        offset=vslot.offset + 1,
                        ap=[[VW * NKV, 128], [65, GH], [1, 64]],
                    )
                    nc.scalar.copy(out=vslot_ap, in_=ps[:])

            # ---- Phase 2: attention + out-projection, sq-pair by sq-pair ----
            with (
                tc.tile_pool(name="mask", bufs=3) as mask_pool,
                tc.tile_pool(name="pt", bufs=2) as pt_pool,
                tc.tile_pool(name="sc", bufs=2, space="PSUM") as sc_pool,
                tc.tile_pool(name="cx", bufs=2, space="PSUM") as cx_pool,
                tc.tile_pool(name="ctx", bufs=2) as ctx_pool,
                tc.tile_pool(name="ost", bufs=3) as ost_pool,
                tc.tile_pool(name="sm", bufs=4) as sm_pool,
            ):
                for p in range(NPAIR):
                    p0 = p * PW
                    masks = []
                    for q in range(PW // MW):
                        mt = mask_pool.tile([128, NKV, MW], BF16, tag="mask")
                        nc.sync.dma_start(
                            out=mt[:],
                            in_=maskt_in.ap().rearrange("(c p) q -> p c q", p=128)[
                                :, :, p0 + q * MW : p0 + (q + 1) * MW
                            ],
                        )
                        masks.append(mt)
                    ctxT = ctx_pool.tile([128, 2, PW], BF16, tag="ctxT")
                    for h in range(GH):
                        hp = (h % 2) * 64
                        j = h // 2
                        pt = pt_pool.tile([128, NKV, PW], BF16, tag="pt")
                        for g in range(NKV // G):
                            ps = sc_pool.tile([128, G * PW], F32, tag="sc")
                            for ci in range(G):
                                c = g * G + ci
                                nc.tensor.matmul(
                                    ps[:, ci * PW : (ci + 1) * PW],
                                    kz[:, h, c * 128 : (c + 1) * 128],
                                    qT[:, j, p0 : p0 + PW],
                                    start=True,
                                    stop=True,
                                )
                            nc.scalar.activation(
                                pt[:, g * G : (g + 1) * G, :],
                                ps[:],
                                mybir.ActivationFunctionType.Exp,
                                scale=INV_SQRT_HD,
                            )
                        for q in range(PW // MW):
                            nc.vector.tensor_tensor(
                                pt[:, :, q * MW : (q + 1) * MW],
                                pt[:, :, q * MW : (q + 1) * MW],
                                masks[q][:],
                                mybir.AluOpType.mult,
                            )
                        # AV: ctx[sq 128, 65] = sum_c P^T[:,c,m].T @ V[:,c,h]
                        for m in range(PW // 128):
                            cx = cx_pool.tile([128, 65], F32, tag="cx")
                            for c in range(NKV):
                                nc.tensor.matmul(
                                    cx[:],
                                    pt[:, c, m * 128 : (m + 1) * 128],
                                    V[:, c, h * 65 : (h + 1) * 65],
                                    start=(c == 0),
                                    stop=(c == NKV - 1),
                                )
                            rec = sm_pool.tile([128, 1], F32, tag="rec")
                            nc.vector.reciprocal(rec[:], cx[:, 0:1])
                            ctxn = sm_pool.tile([128, 64], BF16, tag="ctxn")
                            nc.vector.tensor_scalar_mul(ctxn[:], cx[:, 1:65], rec[:])
                            ctp = cx_pool.tile([128, 128], BF16, tag="ctp")
                            nc.tensor.transpose(
                                ctp[hp : hp + 64, :], ctxn[:], identb[:]
                            )
                            nc.vector.tensor_copy(
                                ctxT[hp : hp + 64, j, m * 128 : (m + 1) * 128],
                                ctp[hp : hp + 64, :],
                            )
                    # out partial for this pair
                    for m in range(PW // 128):
                        for n in range(DIM // 512):
                            po = cx_pool.tile([128, 512], F32, tag="ctp")
                            for j in range(2):
                                nc.tensor.matmul(
                                    po[:],
                                    ctxT[:, j, m * 128 : (m + 1) * 128],
                                    wo_sb[:, j, n * 512 : (n + 1) * 512],
                                    start=(j == 0),
                                    stop=(j == 1),
                                )
                            ost = ost_pool.tile([128, 512], F32, tag="ost")
                            nc.vector.tensor_copy(ost[:], po[:])
                            nc.sync.dma_start(
                                out=out_out[
                                    p0 + m * 128 : p0 + (m + 1) * 128,
                                    n * 512 : (n + 1) * 512,
                                ],
                                in_=ost[:],
                            )

    nc.finalize()
    return nc


_W = {}


def _prep_inputs(x, k_cache, v_cache, mask):
    """Host-side sharding + layout prep. Returns in_maps for 8 cores."""
    ident = np.eye(128, dtype=np.float32)
    identb = np.eye(128, dtype=ml_dtypes.bfloat16)
    keep_t = [
        np.ascontiguousarray((1.0 - mask[b]).T.astype(ml_dtypes.bfloat16))
        for b in range(B)
    ]
    xs = [np.ascontiguousarray(x[b]) for b in range(B)]
    in_maps = []
    for c in range(8):
        b, hg = divmod(c, HG)
        sl = slice(hg * GD, (hg + 1) * GD)
        kts = k_cache[b, :, sl].T.astype(ml_dtypes.bfloat16)  # [256, SC]
        ktc = np.zeros((128, HG, SC), dtype=ml_dtypes.bfloat16)
        for h in range(HG):
            hp2 = (h % 2) * 64
            ktc[hp2 : hp2 + 64, h, :] = kts[h * 64 : (h + 1) * 64, :]
        vaug = np.zeros((SKV, VW), dtype=ml_dtypes.bfloat16)
        vaug[:, 0:VW:65] = 1.0
        vc = v_cache[b, :, sl].astype(ml_dtypes.bfloat16)
        for h in range(GH):
            vaug[:SC, h * 65 + 1 : h * 65 + 65] = vc[:, h * 64 : (h + 1) * 64]
        in_maps.append(
            {
                "x": xs[b],
                "ktc": ktc,
                "vaug": vaug,
                "maskt": keep_t[b],
                "wq": np.ascontiguousarray(_W["Wq"][:, sl]),
                "wk": np.ascontiguousarray(_W["Wk"][:, sl]),
                "wv": np.ascontiguousarray(_W["Wv"][:, sl]),
                "wo": np.ascontiguousarray(_W["Wo"][sl, :]).astype(
                    ml_dtypes.bfloat16
                ),
                "ident": ident,
                "identb": identb,
            }
        )
    return in_maps


def kernel(x, k_cache, v_cache, mask, Wq, bq, Wk, bk, Wv, bv, Wo, bo, _trace=False):
    global _compiled_nc
    x = np.asarray(x)
    k_cache = np.asarray(k_cache)
    v_cache = np.asarray(v_cache)
    mask = np.asarray(mask)
    _W.update(
        Wq=np.asarray(Wq), Wk=np.asarray(Wk), Wv=np.asarray(Wv), Wo=np.asarray(Wo)
    )

    if _compiled_nc is None:
        _compiled_nc = build_kernel()
    nc = _compiled_nc

    in_maps = _prep_inputs(x, k_cache, v_cache, mask)
    res = bass_utils.run_bass_kernel_spmd(
        nc, in_maps, core_ids=list(range(8)), trace=_trace
    )
    kernel.last_results = res

    out = np.zeros((B, SQ, DIM), dtype=np.float32)
    k = np.empty((B, SKV, DIM), dtype=np.float32)
    v = np.empty((B, SKV, DIM), dtype=np.float32)
    k[:, :SC, :] = k_cache
    v[:, :SC, :] = v_cache
    for c in range(8):
        b, hg = divmod(c, HG)
        sl = slice(hg * GD, (hg + 1) * GD)
        r = res.results[c]
        out[b] += r["out"]
        k[b, SC:, sl] = r["ktn"].reshape(GD, SQ).T
        v[b, SC:, sl] = r["vn"]
    # biases are structurally zero in this problem; added for contract parity
    out += np.asarray(bo)[None, None, :]
    k[:, SC:, :] += np.asarray(bk)[None, None, :]
    v[:, SC:, :] += np.asarray(bv)[None, None, :]
    return out, k, v
